# revision 1
# baseline (speedup 1.0000x reference)
"""GATv2 convolution on 8 Trainium2 NeuronCores (Bass/Tile).

Strategy (edge-parallel by target-node range):
  - Host: shard edges by tgt//NSLICE so each core owns all edges of its
    node slice; sort by (tile, src>=32768), pad each 128-node tile's edge
    list to uniform slot counts so one SPMD program fits all cores.
  - Device phase A: T1 = x @ w1 (full table, replicated compute) and
    T2s = x_slice @ w2 (own slice only) via PE-transpose + matmul,
    grouped 4 node-tiles per DMA to amortize HWDGE sequencer cost.
  - Device phase B (per 128-node tile): batch-gather T1[src]/T2s[tgt_loc]
    rows via gpsimd.dma_gather (int16 indices; src split lo/hi around
    32768; <=1024 idxs per call), z = ti + tj, leaky_relu via one DVE
    scalar_tensor_tensor, e = z' * a, per-head sums via strided DVE
    reduce, w = exp(logits) on ACT. Segment softmax-weighted scatter-sum
    = one-hot matmuls accumulated in PSUM ([numerator | denominator] in
    one [128,136] tile), then per-node normalize num/den. No collectives.
"""

import os
import sys

sys.path.insert(0, "/opt/trn_rl_repo")

import numpy as np
import ml_dtypes

import concourse.bass as bass
import concourse.bacc as bacc
import concourse.mybir as mybir
import concourse.tile as tile
from concourse import bass_utils
from concourse.masks import make_identity

P = 128
CORES = 8
HALF = 32768
ALPHA = 0.3
NH = 8
OC = 16

f32 = mybir.dt.float32
bf16 = mybir.dt.bfloat16
i16 = mybir.dt.int16

_last_results = None  # test harness reads exec_time_ns from here


def _roundup(v, m):
    return (v + m - 1) // m * m


def _wrap16(arr):
    """[..., n] int -> [..., 128, n//16] int16 in dma_gather's wrapped layout:
    index i lives at partition i%16, slot i//16, replicated to 128 partitions."""
    *lead, n = arr.shape
    w = arr.reshape(*lead, n // 16, 16)
    w = np.swapaxes(w, -1, -2)  # [..., 16, n//16]
    w = np.tile(w, (*([1] * len(lead)), 8, 1))  # [..., 128, n//16]
    return np.ascontiguousarray(w.astype(np.int16))


def _host_prep(x, w1, w2, a, src, tgt, td_np):
    N, CH = x.shape
    E = src.shape[0]
    assert CH == 128 and N % CORES == 0
    nslice = N // CORES
    nt_b = _roundup(nslice, P) // P

    src = src.astype(np.int64)
    tgt = tgt.astype(np.int64)
    core = tgt // nslice
    tloc = tgt - core * nslice
    tile_i = tloc // P
    loc = tloc % P
    is_hi = (src >= HALF).astype(np.int64)

    ngroups = CORES * nt_b * 2
    key = (core * nt_b + tile_i) * 2 + is_hi
    order = np.argsort(key, kind="stable")
    key_s = key[order]
    src_s = src[order]
    tloc_s = tloc[order]
    loc_s = loc[order]
    core_s = core[order]
    tile_s = tile_i[order]
    hi_s = is_hi[order]

    counts = np.bincount(key, minlength=ngroups).reshape(CORES, nt_b, 2)
    n_lo = counts[:, :, 0]
    n_hi = counts[:, :, 1]
    s_lo = int(_roundup(max(int(n_lo.max()), 16), P))
    s_hi = int(_roundup(max(int(n_hi.max()), 16), P))
    ts = s_lo + s_hi

    gstart = np.zeros(ngroups, dtype=np.int64)
    gstart[1:] = np.cumsum(counts.ravel())[:-1]
    rank = np.arange(E, dtype=np.int64) - gstart[key_s]
    slot = rank + np.where(hi_s == 1, s_lo, 0)

    src_arr = np.zeros((CORES, nt_b, ts), dtype=np.int64)
    tgl_arr = np.full((CORES, nt_b, ts), -1.0, dtype=np.float32)
    tl_arr = np.zeros((CORES, nt_b, ts), dtype=np.int64)
    src_arr[core_s, tile_s, slot] = np.where(hi_s == 1, src_s - HALF, src_s)
    tgl_arr[core_s, tile_s, slot] = loc_s.astype(np.float32)
    tl_arr[core_s, tile_s, slot] = tloc_s

    slo = _wrap16(src_arr[:, :, :s_lo])
    shi = _wrap16(src_arr[:, :, s_lo:])
    tlo = _wrap16(tl_arr)
    # tgtloc column-major per 128-edge chunk: [.., 128, T_C]
    t_c = ts // P
    tgl = np.ascontiguousarray(
        tgl_arr.reshape(CORES, nt_b, t_c, P).transpose(0, 1, 3, 2)
    ).astype(ml_dtypes.bfloat16)

    n_pad = _roundup(N, P)
    xbf = x.astype(ml_dtypes.bfloat16)
    x_pad = np.zeros((n_pad, CH), dtype=ml_dtypes.bfloat16)
    x_pad[:N] = xbf
    xs_pad = np.zeros((CORES, nt_b * P, CH), dtype=ml_dtypes.bfloat16)
    for c in range(CORES):
        xs_pad[c, :nslice] = xbf[c * nslice : (c + 1) * nslice]
    w12 = np.concatenate([w1, w2], axis=1).astype(np.float32)
    a_bc = np.tile(a.reshape(1, CH).astype(np.float32), (P, 1)).astype(td_np)
    iota = np.tile(np.arange(P, dtype=np.float32)[None, :], (P, 1)).astype(
        ml_dtypes.bfloat16
    )

    tglr = tgl_arr.astype(ml_dtypes.bfloat16).reshape(CORES, nt_b, 1, ts)
    iop = np.tile(np.arange(P, dtype=np.float32)[:, None], (1, P)).astype(ml_dtypes.bfloat16)
    in_maps = []
    for c in range(CORES):
        in_maps.append(
            {
                "x": x_pad,
                "xs": np.ascontiguousarray(xs_pad[c]),
                "w12": w12,
                "a_bc": a_bc,
                "iota": iota,
                "iop": iop,
                "slo": np.ascontiguousarray(slo[c]),
                "shi": np.ascontiguousarray(shi[c]),
                "tlo": np.ascontiguousarray(tlo[c]),
                "tgl": np.ascontiguousarray(tgl[c]),
                "tglr": np.ascontiguousarray(tglr[c]),
            }
        )
    dims = dict(
        N=N, CH=CH, nslice=nslice, nt_b=nt_b, n_pad=n_pad,
        s_lo=s_lo, s_hi=s_hi, ts=ts, t_c=t_c,
    )
    return in_maps, dims


def _build_program(dims, td=bf16):
    N = dims["N"]
    CH = dims["CH"]
    nslice = dims["nslice"]
    nt_b = dims["nt_b"]
    n_pad = dims["n_pad"]
    s_lo = dims["s_lo"]
    s_hi = dims["s_hi"]
    ts = dims["ts"]
    t_c = dims["t_c"]
    nt_a = n_pad // P
    xsr = nt_b * P
    kstage = os.environ.get("KSTAGE", "full")

    nc = bacc.Bacc("TRN2", target_bir_lowering=False, debug=False,
                   num_devices=CORES)

    x_in = nc.dram_tensor("x", [n_pad, CH], bf16, kind="ExternalInput")
    xs_in = nc.dram_tensor("xs", [xsr, CH], bf16, kind="ExternalInput")
    w12_in = nc.dram_tensor("w12", [CH, 2 * CH], f32, kind="ExternalInput")
    abc_in = nc.dram_tensor("a_bc", [P, CH], td, kind="ExternalInput")
    iota_in = nc.dram_tensor("iota", [P, P], bf16, kind="ExternalInput")
    slo_in = nc.dram_tensor("slo", [nt_b, P, s_lo // 16], i16, kind="ExternalInput")
    shi_in = nc.dram_tensor("shi", [nt_b, P, s_hi // 16], i16, kind="ExternalInput")
    kexp = os.environ.get("KEXP", "1") == "1"
    if not kexp:
        tlo_in = nc.dram_tensor("tlo", [nt_b, P, ts // 16], i16,
                                kind="ExternalInput")
    tgl_in = nc.dram_tensor("tgl", [nt_b, P, t_c], bf16, kind="ExternalInput")
    if kexp:
        tglr_in = nc.dram_tensor("tglr", [nt_b, 1, ts], bf16,
                                 kind="ExternalInput")
        iop_in = nc.dram_tensor("iop", [P, P], bf16, kind="ExternalInput")
    out = nc.dram_tensor("out", [nslice, CH], f32, kind="ExternalOutput")
    # tables must be standalone dram tensors (offset 0): dma_gather from a
    # DRAM *pool tile* (nonzero offset in the pool arena) crashes the Q7
    t1 = nc.dram_tensor("t1tab", [n_pad, CH], td, kind="Internal")
    t2 = nc.dram_tensor("t2tab", [xsr, CH], td, kind="Internal")

    with tile.TileContext(nc) as tc:
        with tc.tile_pool(name="const", bufs=1) as cp:
            ident = cp.tile([P, P], f32)
            make_identity(nc, ident[:])
            identb = cp.tile([P, P], bf16)
            nc.vector.tensor_copy(out=identb[:], in_=ident[:])
            w12f = cp.tile([CH, 2 * CH], f32)
            nc.sync.dma_start(out=w12f[:], in_=w12_in[:])
            if td == f32:
                w12t = w12f
            else:
                w12t = cp.tile([CH, 2 * CH], td)
                nc.vector.tensor_copy(out=w12t[:], in_=w12f[:])
            a_t = cp.tile([P, CH], td)
            nc.sync.dma_start(out=a_t[:], in_=abc_in[:])
            iota_t = cp.tile([P, P], bf16)
            nc.sync.dma_start(out=iota_t[:], in_=iota_in[:])
            if kexp:
                iop_t = cp.tile([P, P], bf16, tag="ioptile")
                nc.sync.dma_start(out=iop_t[:], in_=iop_in[:])

            # ---------------- Phase A: projection tables ----------------
            # groups of 4 node tiles per DMA to amortize HWDGE seq cost
            with (
                tc.tile_pool(name="pa", bufs=3) as pa,
                tc.tile_pool(name="pa_ps", bufs=2, space="PSUM") as pa_ps,
                tc.tile_pool(name="pa_ps2", bufs=2, space="PSUM") as pa_ps2,
            ):
                def project_group(src_dram, dst_tab, w_sl, base, ntile, eng):
                    rows = ntile * P
                    src4 = src_dram[base * P : base * P + rows, :].rearrange(
                        "(k p) c -> p k c", p=P
                    )
                    xt4 = pa.tile([P, ntile, CH], bf16, tag="xt")
                    nc.sync.dma_start(out=xt4[:], in_=src4)
                    psT = pa_ps.tile([P, ntile * P], bf16, space="PSUM", tag="psT")
                    for k in range(ntile):
                        nc.tensor.transpose(
                            out=psT[:, k * P : (k + 1) * P],
                            in_=xt4[:, k, :],
                            identity=identb[:],
                        )
                    xT = pa.tile([P, ntile * P], td, tag="xT")
                    if eng == 0:
                        nc.vector.tensor_copy(out=xT[:], in_=psT[:])
                    else:
                        nc.scalar.copy(out=xT[:], in_=psT[:])
                    mm = pa_ps2.tile([P, ntile * CH], f32, space="PSUM", tag="mm")
                    for k in range(ntile):
                        nc.tensor.matmul(
                            out=mm[:, k * CH : (k + 1) * CH],
                            lhsT=xT[:, k * P : (k + 1) * P],
                            rhs=w_sl,
                            start=True,
                            stop=True,
                        )
                    o = pa.tile([P, ntile * CH], td, tag="o")
                    if eng == 0:
                        nc.scalar.copy(out=o[:], in_=mm[:])
                    else:
                        nc.vector.tensor_copy(out=o[:], in_=mm[:])
                    dst4 = dst_tab[base * P : base * P + rows, :].rearrange(
                        "(k p) c -> p k c", p=P
                    )
                    o_v = o[:].rearrange("p (k c) -> p k c", c=CH)
                    nc.scalar.dma_start(out=dst4, in_=o_v)

                if kstage != "IO":
                    G4 = int(os.environ.get("G4", "4"))
                    gi = 0
                    for base in range(0, nt_a, G4):
                        nt = min(G4, nt_a - base)
                        project_group(x_in, t1, w12t[:, 0:CH], base, nt, gi % 2)
                        gi += 1
                    for base in range(0, nt_b, G4):
                        nt = min(G4, nt_b - base)
                        project_group(xs_in, t2, w12t[:, CH : 2 * CH], base, nt,
                                      gi % 2)
                        gi += 1

            # ---------------- Phase B: edge processing ----------------
            _pbb = int(os.environ.get("PB_BUFS", "2"))
            _pgb = int(os.environ.get("PG_BUFS", "2"))
            with (
                tc.tile_pool(name="pb", bufs=_pbb) as pb,
                tc.tile_pool(name="pbg", bufs=_pgb) as pbg,
                tc.tile_pool(name="pb_ps", bufs=2, space="PSUM") as pb_ps,
            ):
                for t in range(nt_b):
                    if kstage == "IO":
                        if t == 0:
                            zt0 = pb.tile([P, CH], f32, tag="zt")
                            nc.vector.tensor_copy(out=zt0[:], in_=ident[:])
                            nc.sync.dma_start(out=out[0:P, :], in_=zt0[:])
                        continue
                    if kstage == "A":
                        zt = pb.tile([P, CH], f32, tag="zt")
                        nc.vector.tensor_copy(out=zt[:], in_=ident[:])
                        rows = min(P, nslice - t * P)
                        nc.sync.dma_start(
                            out=out[t * P : t * P + rows, :], in_=zt[:rows, :]
                        )
                        continue
                    li = pbg.tile([P, s_lo // 16], i16, tag="li")
                    nc.sync.dma_start(out=li[:], in_=slo_in[t])
                    hi = pbg.tile([P, s_hi // 16], i16, tag="hi")
                    nc.scalar.dma_start(out=hi[:], in_=shi_in[t])
                    if not kexp:
                        tli = pb.tile([P, ts // 16], i16, tag="tli")
                        nc.sync.dma_start(out=tli[:], in_=tlo_in[t])
                    tg = pb.tile([P, t_c], bf16, tag="tg")
                    nc.scalar.dma_start(out=tg[:], in_=tgl_in[t])

                    GMAX = 1024  # dma_gather crashes above 1024 idxs/call

                    def gather_split(dst, dst_off, src_ap, idx_tile, n):
                        for off in range(0, n, GMAX):
                            sz = min(GMAX, n - off)
                            o = dst_off + off
                            nc.gpsimd.dma_gather(
                                out_ap=dst[:, o // P : (o + sz) // P, :],
                                in_ap=src_ap,
                                idxs_ap=idx_tile[:, off // 16 : (off + sz) // 16],
                                num_idxs=sz,
                                num_idxs_reg=sz,
                                elem_size=CH,
                                single_packet=os.environ.get("KSP", "0")
                                == "1",
                            )

                    g1 = pbg.tile([P, t_c, P], td, tag="g1")  # ti = T1[src]
                    gather_split(g1, 0, t1[:], li, s_lo)
                    if kstage == "G1":
                        zt = pb.tile([P, CH], f32, tag="zt")
                        nc.vector.tensor_copy(out=zt[:], in_=g1[:, 0, :])
                        rows = min(P, nslice - t * P)
                        nc.sync.dma_start(
                            out=out[t * P : t * P + rows, :], in_=zt[:rows, :]
                        )
                        continue
                    hi_src = t1[HALF:, :] if N > HALF else t1[:]
                    gather_split(g1, s_lo, hi_src, hi, s_hi)
                    g2 = pb.tile([P, t_c, P], td, tag="g2")  # tj = T2s[tloc]
                    if kexp:
                        # tj via one-hot expand matmul from the tile's own
                        # 128 T2 rows (tgt is sorted; no gather needed)
                        t2til = pb.tile([P, CH], td, tag="t2til")
                        nc.scalar.dma_start(out=t2til[:],
                                            in_=t2[t * P : (t + 1) * P, :])
                        tgr = pbg.tile([P, ts], bf16, tag="tgr")
                        nc.sync.dma_start(
                            out=tgr[:],
                            in_=tglr_in[t].broadcast_to([P, ts]),
                        )
                        ohT = pb.tile([P, ts], bf16, tag="ohT")
                        nc.vector.tensor_tensor(
                            out=ohT[:].rearrange("p (k j) -> p k j", j=P),
                            in0=iop_t[:][:, None, :].broadcast_to([P, t_c, P]),
                            in1=tgr[:].rearrange("p (k j) -> p k j", j=P),
                            op=mybir.AluOpType.is_equal,
                        )
                        EXG = 8  # psum group: 8 chunks = 2 banks
                        for g0 in range(0, t_c, EXG):
                            gn = min(EXG, t_c - g0)
                            hjp = pb_ps.tile([P, EXG * P], f32, space="PSUM",
                                             tag="hjp")
                            for kk in range(gn):
                                k = g0 + kk
                                nc.tensor.matmul(
                                    out=hjp[:, kk * P : (kk + 1) * P],
                                    lhsT=ohT[:, k * P : (k + 1) * P],
                                    rhs=t2til[:],
                                    start=True,
                                    stop=True,
                                )
                            nc.scalar.copy(
                                out=g2[:, g0 : g0 + gn, :].rearrange(
                                    "p a b -> p (a b)"
                                ),
                                in_=hjp[:, : gn * P],
                            )
                    else:
                        gather_split(g2, 0, t2[:], tli, ts)

                    if kstage == "G":
                        zt = pb.tile([P, CH], f32, tag="zt")
                        nc.vector.tensor_tensor(
                            out=zt[:], in0=g1[:, 0, :], in1=g2[:, 0, :],
                            op=mybir.AluOpType.add,
                        )
                        rows = min(P, nslice - t * P)
                        nc.sync.dma_start(
                            out=out[t * P : t * P + rows, :], in_=zt[:rows, :]
                        )
                        continue

                    oh = pb.tile([P, ts], bf16, tag="oh")
                    nc.vector.tensor_tensor(
                        out=oh[:].rearrange("p (k n) -> p k n", n=P),
                        in0=tg[:][:, :, None].broadcast_to([P, t_c, P]),
                        in1=iota_t[:][:, None, :].broadcast_to([P, t_c, P]),
                        op=mybir.AluOpType.is_equal,
                    )

                    g1f = g1[:].rearrange("p a b -> p (a b)")
                    g2f = g2[:].rearrange("p a b -> p (a b)")
                    z = pb.tile([P, ts], td, tag="z")
                    nc.vector.tensor_tensor(out=z[:], in0=g1f, in1=g2f,
                                            op=mybir.AluOpType.add)
                    # leaky_relu = max(0.3*z, z); write into g2 (tj dead)
                    zp = g2f
                    nc.vector.scalar_tensor_tensor(
                        out=zp, in0=z[:], scalar=ALPHA, in1=z[:],
                        op0=mybir.AluOpType.mult, op1=mybir.AluOpType.max,
                    )
                    # e = z' * a  (a broadcast over chunks); write into z
                    ew = z[:]
                    nc.vector.tensor_tensor(
                        out=ew.rearrange("p (k c) -> p k c", c=CH),
                        in0=zp.rearrange("p (k c) -> p k c", c=CH),
                        in1=a_t[:][:, None, :].broadcast_to([P, t_c, CH]),
                        op=mybir.AluOpType.mult,
                    )
                    # logits[e, k, h] = sum_c e[k, h, c]
                    lg = pb.tile([P, t_c * NH], f32, tag="lg")
                    nc.vector.tensor_reduce(
                        out=lg[:].rearrange("p (k h) -> p k h", h=NH),
                        in_=ew.rearrange("p (k h c) -> p k h c", h=NH, c=OC),
                        axis=mybir.AxisListType.X,
                        op=mybir.AluOpType.add,
                    )
                    # scat[e, k, :] = [msg(128) | w(8)] in bf16
                    scat = pb.tile([P, t_c * 136], bf16, tag="scat")
                    scat_r = scat[:].rearrange("p (k c) -> p k c", c=136)
                    nc.scalar.activation(
                        out=scat_r[:, :, CH : CH + NH],
                        in_=lg[:].rearrange("p (k h) -> p k h", h=NH),
                        func=mybir.ActivationFunctionType.Exp,
                    )
                    w_bc = scat_r[:, :, CH : CH + NH][:, :, :, None].broadcast_to(
                        [P, t_c, NH, OC]
                    )
                    nc.vector.tensor_tensor(
                        out=scat_r[:, :, 0:CH].rearrange(
                            "p k (h c) -> p k h c", c=OC
                        ),
                        in0=g1[:].rearrange("p k (h c) -> p k h c", c=OC),
                        in1=w_bc,
                        op=mybir.AluOpType.mult,
                    )

                    acc_ps = pb_ps.tile([P, 136], f32, space="PSUM", tag="acc")
                    for k in range(t_c):
                        nc.tensor.matmul(
                            out=acc_ps[:],
                            lhsT=oh[:, k * P : (k + 1) * P],
                            rhs=scat[:, k * 136 : (k + 1) * 136],
                            start=(k == 0),
                            stop=(k == t_c - 1),
                        )

                    acc = pb.tile([P, 136], f32, tag="accs")
                    nc.scalar.copy(out=acc[:], in_=acc_ps[:])
                    dg = pb.tile([P, NH], f32, tag="dg")
                    nc.vector.tensor_scalar_max(
                        out=dg[:], in0=acc[:, CH : CH + NH], scalar1=1e-30
                    )
                    rc = pb.tile([P, NH], f32, tag="rc")
                    nc.vector.reciprocal(out=rc[:], in_=dg[:])
                    ot = pb.tile([P, CH], f32, tag="ot")
                    nc.vector.tensor_tensor(
                        out=ot[:].rearrange("p (h c) -> p h c", c=OC),
                        in0=acc[:, 0:CH].rearrange("p (h c) -> p h c", c=OC),
                        in1=rc[:][:, :, None].broadcast_to([P, NH, OC]),
                        op=mybir.AluOpType.mult,
                    )
                    rows = min(P, nslice - t * P)
                    nc.sync.dma_start(
                        out=out[t * P : t * P + rows, :], in_=ot[:rows, :]
                    )

    nc.compile()
    return nc


def _td_np(td):
    return ml_dtypes.bfloat16 if td == bf16 else np.float32


def kernel(x, w1, w2, a, src, tgt):
    global _last_results
    x = np.asarray(x, dtype=np.float32)
    w1 = np.asarray(w1, dtype=np.float32)
    w2 = np.asarray(w2, dtype=np.float32)
    a = np.asarray(a, dtype=np.float32)
    src = np.asarray(src)
    tgt = np.asarray(tgt)

    td = f32 if os.environ.get("KTBL", "bf16") == "f32" else bf16
    in_maps, dims = _host_prep(x, w1, w2, a, src, tgt, _td_np(td))
    nc = _build_program(dims, td=td)

    trace = bool(os.environ.get("KBENCH_TRACE"))
    res = bass_utils.run_bass_kernel_spmd(
        nc, in_maps, core_ids=list(range(CORES)), trace=trace
    )
    _last_results = res
    nslice = dims["nslice"]
    out = np.empty((x.shape[0], x.shape[1]), dtype=np.float32)
    for c in range(CORES):
        out[c * nslice : (c + 1) * nslice] = res.results[c]["out"]
    return out



# revision 29
# speedup vs baseline: 4.5908x; 4.5908x over previous
"""GATv2 convolution on 8 Trainium2 NeuronCores (Bass/Tile) — v2.

Strategy (edge-parallel by target-node range):
  Host: shard edges by tgt//nslice so each core owns all edges of its node
  slice; within a core, group edges by 128-node tile and sort tiles by edge
  count (descending) so tile-index k has a similar count on every core
  (shrinks SPMD padding). Per tile, split src indices around 32768 (int16
  gather limit), pad each (lo, hi) slot range to a multiple of 128.

  Device phase A: T1 = x @ w1 (full table, every core) and T2 = xs @ w2
  (own permuted slice) via DMA-transpose loads + PE matmul.

  Device phase B (per node tile):
   - gpsimd.dma_gather T1[src] rows (bf16, <=1024 idxs/call) -> g1 [e,128].
   - ohT (node-part one-hot, DVE is_equal 2x from DMA-broadcast tgt-locs),
     oh (edge-part one-hot fp16, DVE is_equal from col-major tgt-locs).
   - PE: z = ohT.T @ t2til (+ identity-matmul accumulate of g1) in PSUM.
   - ACT: zp = LeakyRelu(z) PSUM->SBUF fp16 (fused evacuation).
   - DVE fp16 2x: ew = zp*a, logits = segment-reduce(ew).
   - ACT: wexp = Exp(logits) broadcast-expanded to [e,128]; w cols = Exp.
   - DVE: msg = g1 * wexp -> scat [msg|w] fp16.
   - PE: acc[node,136] += oh.T @ scat per chunk; normalize num/den; DMA out.
  Host un-permutes output rows. No collectives.
"""

import os
import sys

sys.path.insert(0, "/opt/trn_rl_repo")

import numpy as np
import ml_dtypes

import concourse.bass as bass
import concourse.bacc as bacc
import concourse.mybir as mybir
import concourse.tile as tile
from concourse import bass_utils
from concourse.masks import make_identity

P = 128
CORES = 8
HALF = 32768
ALPHA = 0.3
NH = 8
OC = 16

f32 = mybir.dt.float32
bf16 = mybir.dt.bfloat16
fp16 = mybir.dt.float16
i16 = mybir.dt.int16

_last_results = None  # test harness reads exec_time_ns from here


def _roundup(v, m):
    return (v + m - 1) // m * m


def _wrap16(arr):
    """[..., n] int -> [..., 128, n//16] int16 in dma_gather's wrapped layout:
    index i lives at partition i%16, slot i//16, replicated to 128 partitions."""
    *lead, n = arr.shape
    w = arr.reshape(*lead, n // 16, 16)
    w = np.swapaxes(w, -1, -2)  # [..., 16, n//16]
    w = np.tile(w, (*([1] * len(lead)), 8, 1))  # [..., 128, n//16]
    return np.ascontiguousarray(w.astype(np.int16))


def _host_prep(x, w1, w2, a, src, tgt):
    N, CH = x.shape
    E = src.shape[0]
    assert CH == 128 and N % CORES == 0
    nslice = N // CORES
    nt = _roundup(nslice, P) // P

    src = src.astype(np.int64)
    tgt = tgt.astype(np.int64)
    core = tgt // nslice
    tloc = tgt - core * nslice
    tile_i = tloc // P
    loc = tloc % P
    is_hi = (src >= HALF).astype(np.int64)

    # counts per (core, tile, half)
    key_full = (core * nt + tile_i) * 2 + is_hi
    counts = np.bincount(key_full, minlength=CORES * nt * 2).reshape(CORES, nt, 2)
    tot = counts.sum(axis=2)  # [CORES, nt]

    # per-core tile permutation: slot k holds the k-th busiest tile
    perm = np.argsort(-tot, axis=1, kind="stable")  # [CORES, nt]
    rank = np.empty_like(perm)
    for c in range(CORES):
        rank[c, perm[c]] = np.arange(nt)

    lo_sorted = np.take_along_axis(counts[:, :, 0], perm, axis=1)
    hi_sorted = np.take_along_axis(counts[:, :, 1], perm, axis=1)
    s_lo = np.maximum(_roundup(lo_sorted.max(axis=0), P), P).astype(np.int64)
    s_hi = _roundup(hi_sorted.max(axis=0), P).astype(np.int64)
    ts_k = s_lo + s_hi
    tc_k = ts_k // P

    off_ts = np.zeros(nt + 1, dtype=np.int64)
    off_ts[1:] = np.cumsum(ts_k)
    off_lo = np.zeros(nt + 1, dtype=np.int64)
    off_lo[1:] = np.cumsum(s_lo)
    off_hi = np.zeros(nt + 1, dtype=np.int64)
    off_hi[1:] = np.cumsum(s_hi)
    TS = int(off_ts[-1])
    SLO = int(off_lo[-1])
    SHI = int(off_hi[-1])

    # slot within the tile's padded range
    k_of_edge = rank[core, tile_i]  # tile slot per edge
    skey = (core * nt + k_of_edge) * 2 + is_hi
    order = np.argsort(skey, kind="stable")
    gstart = np.zeros(CORES * nt * 2, dtype=np.int64)
    cnt_s = np.bincount(skey, minlength=CORES * nt * 2)
    gstart[1:] = np.cumsum(cnt_s)[:-1]
    rank_e = np.arange(E, dtype=np.int64) - gstart[skey[order]]

    core_s = core[order]
    k_s = k_of_edge[order]
    hi_s = is_hi[order]
    src_s = src[order]
    loc_s = loc[order]

    slot = rank_e + np.where(hi_s == 1, s_lo[k_s], 0)

    src_lo_arr = np.zeros((CORES, SLO), dtype=np.int64)
    src_hi_arr = np.zeros((CORES, SHI), dtype=np.int64)
    tgl_arr = np.full((CORES, TS), -1.0, dtype=np.float32)

    lo_m = hi_s == 0
    hi_m = ~lo_m
    src_lo_arr[core_s[lo_m], off_lo[k_s[lo_m]] + slot[lo_m]] = src_s[lo_m]
    src_hi_arr[core_s[hi_m], off_hi[k_s[hi_m]] + slot[hi_m] - s_lo[k_s[hi_m]]] = (
        src_s[hi_m] - HALF
    )
    tgl_arr[core_s, off_ts[k_s] + slot] = loc_s.astype(np.float32)

    slo_w = _wrap16(src_lo_arr)  # [CORES, 128, SLO//16]
    shi_w = _wrap16(src_hi_arr)
    tgr = tgl_arr.astype(ml_dtypes.bfloat16).reshape(CORES, 1, TS)
    # col-major per 128-edge chunk: [CORES, 128, TS//128]
    tg = np.ascontiguousarray(
        tgl_arr.reshape(CORES, TS // P, P).transpose(0, 2, 1)
    ).astype(ml_dtypes.bfloat16)

    n_pad = _roundup(N, P)
    xbf = x.astype(ml_dtypes.bfloat16)
    x_pad = np.zeros((n_pad, CH), dtype=ml_dtypes.bfloat16)
    x_pad[:N] = xbf
    xT = np.ascontiguousarray(x_pad.T)  # [CH, n_pad] host-side transpose

    # xs permuted: slot k of core c holds nodes of tile perm[c,k]
    xs_pad = np.zeros((CORES, nt * P, CH), dtype=ml_dtypes.bfloat16)
    for c in range(CORES):
        for k in range(nt):
            t = perm[c, k]
            rows = min(P, nslice - t * P)
            xs_pad[c, k * P : k * P + rows] = xbf[
                c * nslice + t * P : c * nslice + t * P + rows
            ]
    xsT = np.ascontiguousarray(np.transpose(xs_pad, (0, 2, 1)))  # [C, CH, nt*P]

    w12 = np.concatenate([w1, w2], axis=1).astype(ml_dtypes.bfloat16)
    a_bc = np.tile(a.reshape(1, CH).astype(np.float32), (P, 1)).astype(np.float16)
    iota = np.tile(np.arange(P, dtype=np.float32)[None, :], (P, 1)).astype(
        ml_dtypes.bfloat16
    )
    iop = np.tile(np.arange(P, dtype=np.float32)[:, None], (1, P)).astype(
        ml_dtypes.bfloat16
    )

    in_maps = []
    for c in range(CORES):
        in_maps.append(
            {
                "xT": xT,
                "xsT": np.ascontiguousarray(xsT[c]),
                "w12": w12,
                "a_bc": a_bc,
                "iota": iota,
                "iop": iop,
                "slo": np.ascontiguousarray(slo_w[c]),
                **({"shi": np.ascontiguousarray(shi_w[c])} if SHI else {}),
                "tg": np.ascontiguousarray(tg[c]),
                "tgr": np.ascontiguousarray(tgr[c]),
            }
        )
    dims = dict(
        N=N, CH=CH, nslice=nslice, nt=nt, n_pad=n_pad,
        s_lo=[int(v) for v in s_lo], s_hi=[int(v) for v in s_hi],
        SLO=SLO, SHI=SHI, TS=TS,
    )
    return in_maps, dims, perm


def _unpermute(res_rows, perm, nslice, nt):
    """res_rows: [CORES][nt*P, CH] tile-slot-major -> [N, CH] node order."""
    CH = res_rows[0].shape[1]
    out = np.empty((CORES * nslice, CH), dtype=res_rows[0].dtype)
    for c in range(CORES):
        for k in range(nt):
            t = perm[c, k]
            rows = min(P, nslice - t * P)
            out[c * nslice + t * P : c * nslice + t * P + rows] = res_rows[c][
                k * P : k * P + rows
            ]
    return out


def _build_program(dims):
    CH = dims["CH"]
    nt = dims["nt"]
    n_pad = dims["n_pad"]
    s_lo = dims["s_lo"]
    s_hi = dims["s_hi"]
    SLO = dims["SLO"]
    SHI = dims["SHI"]
    TS = dims["TS"]
    nt_a = n_pad // P
    xsr = nt * P

    GMAX = int(os.environ.get("GMAX", "1024"))
    EXG = 8  # chunks per PSUM expand group (2 banks f32)
    G4 = int(os.environ.get("G4", "8"))
    leaky_act = os.environ.get("KLEAKY", "act") == "act"
    scratch = int(os.environ.get("KSCRATCH", "16384"))

    nc = bacc.Bacc("TRN2", target_bir_lowering=False, debug=False,
                   num_devices=CORES, dynamic_dma_scratch_size=scratch)

    x_in = nc.dram_tensor("xT", [CH, n_pad], bf16, kind="ExternalInput")
    xs_in = nc.dram_tensor("xsT", [CH, xsr], bf16, kind="ExternalInput")
    w12_in = nc.dram_tensor("w12", [CH, 2 * CH], bf16, kind="ExternalInput")
    abc_in = nc.dram_tensor("a_bc", [P, CH], fp16, kind="ExternalInput")
    iota_in = nc.dram_tensor("iota", [P, P], bf16, kind="ExternalInput")
    iop_in = nc.dram_tensor("iop", [P, P], bf16, kind="ExternalInput")
    slo_in = nc.dram_tensor("slo", [P, SLO // 16], i16, kind="ExternalInput")
    shi_in = (nc.dram_tensor("shi", [P, SHI // 16], i16, kind="ExternalInput")
              if SHI else None)
    tg_in = nc.dram_tensor("tg", [P, TS // P], bf16, kind="ExternalInput")
    tgr_in = nc.dram_tensor("tgr", [1, TS], bf16, kind="ExternalInput")
    out = nc.dram_tensor("out", [xsr, CH], f32, kind="ExternalOutput")
    # gather tables must be standalone dram tensors (offset 0): dma_gather
    # from a DRAM pool tile (nonzero offset in the pool arena) crashes the Q7
    t1 = nc.dram_tensor("t1tab", [n_pad, CH], bf16, kind="Internal")
    t2 = nc.dram_tensor("t2tab", [xsr, CH], bf16, kind="Internal")

    off_lo = [0]
    off_hi = [0]
    off_ts = [0]
    for k in range(nt):
        off_lo.append(off_lo[-1] + s_lo[k])
        off_hi.append(off_hi[-1] + s_hi[k])
        off_ts.append(off_ts[-1] + s_lo[k] + s_hi[k])
    TCMAX = max((s_lo[k] + s_hi[k]) // P for k in range(nt))

    with tile.TileContext(nc) as tc:
        with tc.tile_pool(name="const", bufs=1) as cp:
            ident = cp.tile([P, P], f32)
            make_identity(nc, ident[:])
            identb = cp.tile([P, P], bf16)
            nc.vector.tensor_copy(out=identb[:], in_=ident[:])
            w12t = cp.tile([CH, 2 * CH], bf16)
            nc.sync.dma_start(out=w12t[:], in_=w12_in[:])
            a_t = cp.tile([P, CH], fp16)
            nc.sync.dma_start(out=a_t[:], in_=abc_in[:])
            iota_t = cp.tile([P, P], bf16)
            nc.sync.dma_start(out=iota_t[:], in_=iota_in[:])
            iop_t = cp.tile([P, P], bf16)
            nc.sync.dma_start(out=iop_t[:], in_=iop_in[:])
            iopf = cp.tile([P, 1], f32)
            nc.vector.tensor_copy(out=iopf[:], in_=iop_t[:, 0:1])
            # iotexp[p, j*TCMAX + k] = j (j-major iota for the 2x oh build)
            iotexp = cp.tile([P, P * TCMAX], bf16)
            nc.vector.tensor_copy(
                out=iotexp[:].rearrange("p (j k) -> p j k", k=TCMAX),
                in_=iota_t[:][:, :, None].broadcast_to([P, P, TCMAX]),
            )

            # ---------------- Phase A: projection tables ----------------
            # t1 first: phase-B gathers gate on the complete t1 table, while
            # t2 tiles are consumed one at a time (overlaps with phase B)
            with (
                tc.tile_pool(name="pa", bufs=int(os.environ.get("PA_BUFS", "4"))) as pa,
                tc.tile_pool(name="pa_ps", bufs=int(os.environ.get("PA_PS", "3")),
                             space="PSUM") as pa_ps,
            ):
                def project_group(src_dram, dst_tab, w_sl, base, ntile, eng):
                    # phase A stays on the scalar HWDGE queue so phase-B
                    # prefetch loads (sync queue) flow during projection
                    rows = ntile * P
                    xTt = pa.tile([P, rows], bf16, tag="xT")
                    nc.scalar.dma_start(
                        out=xTt[:], in_=src_dram[:, base * P : base * P + rows]
                    )
                    mm = pa_ps.tile([P, rows], f32, space="PSUM", tag="mm")
                    for k in range(ntile):
                        nc.tensor.matmul(
                            out=mm[:, k * P : (k + 1) * P],
                            lhsT=xTt[:, k * P : (k + 1) * P],
                            rhs=w_sl,
                            start=True,
                            stop=True,
                        )
                    o = pa.tile([P, rows], bf16, tag="o")
                    if eng == 0:
                        nc.scalar.copy(out=o[:], in_=mm[:])
                    else:
                        nc.vector.tensor_copy(out=o[:], in_=mm[:])
                    dst4 = dst_tab[base * P : base * P + rows, :].rearrange(
                        "(k p) c -> p k c", p=P
                    )
                    o_v = o[:].rearrange("p (k c) -> p k c", c=CH)
                    nc.scalar.dma_start(out=dst4, in_=o_v)

                gi = 0
                for base in range(0, nt_a, G4):
                    ntile = min(G4, nt_a - base)
                    project_group(x_in, t1, w12t[:, 0:CH], base, ntile, gi % 2)
                    gi += 1
                for base in range(0, nt, G4):
                    ntile = min(G4, nt - base)
                    project_group(xs_in, t2, w12t[:, CH : 2 * CH], base, ntile,
                                  gi % 2)
                    gi += 1

            # ---------------- Phase B: edge processing ----------------
            # Software-pipelined across tiles in three stages so each
            # engine's in-order queue never blocks on a prior tile's late
            # dependencies:
            #   A(t): loads, gathers, one-hot builds, expand+add (PE),
            #         leaky (ACT, PSUM->SBUF)
            #   B(t): ew, logit tree, exp-expand, msg
            #   C(t): scatter matmuls, normalize, out DMA
            _pbb = int(os.environ.get("PB_BUFS", "2"))
            _pcb = int(os.environ.get("PC_BUFS", "3"))
            _pgb = int(os.environ.get("PG_BUFS", "3"))
            _psb = int(os.environ.get("PS_BUFS", "3"))
            with (
                tc.tile_pool(name="pb", bufs=_pbb) as pb,
                tc.tile_pool(name="pc", bufs=_pcb) as pc,
                tc.tile_pool(name="pbg", bufs=_pgb) as pbg,
                tc.tile_pool(name="pb_ps", bufs=_psb, space="PSUM") as pb_ps,
                tc.tile_pool(name="pb_ps2", bufs=2, space="PSUM") as pb_ps2,
            ):
                def stage_a(t):
                    sl, sh = s_lo[t], s_hi[t]
                    ts = sl + sh
                    t_c = ts // P

                    li = pbg.tile([P, sl // 16], i16, tag="li")
                    nc.sync.dma_start(
                        out=li[:], in_=slo_in[:, off_lo[t] // 16 : off_lo[t + 1] // 16]
                    )
                    if sh:
                        hi_t = pbg.tile([P, sh // 16], i16, tag="hi")
                        nc.sync.dma_start(
                            out=hi_t[:],
                            in_=shi_in[:, off_hi[t] // 16 : off_hi[t + 1] // 16],
                        )
                    tg_t = pb.tile([P, t_c], bf16, tag="tg")
                    nc.sync.dma_start(
                        out=tg_t[:], in_=tg_in[:, off_ts[t] // P : off_ts[t + 1] // P]
                    )
                    # loc-row replicate to 128 partitions: alternate between
                    # broadcast-DMA (DMA-engine bytes) and Q7 daisy-chain
                    # broadcast (Pool time) to balance the two resources
                    tgr_t = pbg.tile([P, ts], bf16, tag="tgr")
                    if t % 2 == 0:
                        nc.sync.dma_start(
                            out=tgr_t[0:1, :],
                            in_=tgr_in[0:1, off_ts[t] : off_ts[t + 1]],
                        )
                        nc.gpsimd.partition_broadcast(
                            out_ap=tgr_t[:], in_ap=tgr_t[0:1, :]
                        )
                    else:
                        nc.sync.dma_start(
                            out=tgr_t[:],
                            in_=tgr_in[0:1, off_ts[t] : off_ts[t + 1]]
                            .broadcast_to([P, ts]),
                        )
                    t2til = pb.tile([P, CH], bf16, tag="t2til")
                    nc.scalar.dma_start(out=t2til[:], in_=t2[t * P : (t + 1) * P, :])

                    def gather_split(dst, dst_off, src_ap, idx_tile, n):
                        for off in range(0, n, GMAX):
                            sz = min(GMAX, n - off)
                            o = dst_off + off
                            nc.gpsimd.dma_gather(
                                out_ap=dst[:, o // P : (o + sz) // P, :],
                                in_ap=src_ap,
                                idxs_ap=idx_tile[:, off // 16 : (off + sz) // 16],
                                num_idxs=sz,
                                num_idxs_reg=sz,
                                elem_size=CH,
                                single_packet=os.environ.get("KSP", "0") == "1",
                            )

                    g1 = pbg.tile([P, t_c, P], bf16, tag="g1")
                    gather_split(g1, 0, t1[0 : min(HALF, n_pad), :], li, sl)
                    if sh:
                        gather_split(g1, sl, t1[HALF:, :], hi_t, sh)

                    # one-hots
                    ohT = pb.tile([P, ts], bf16, tag="ohT")
                    nc.vector.tensor_scalar(
                        out=ohT[:],
                        in0=tgr_t[:],
                        scalar1=iopf[:],
                        scalar2=None,
                        op0=mybir.AluOpType.is_equal,
                    )
                    # j-major one-hot (edge-part): last AP dim is the
                    # packed chunk index so the is_equal runs in 2x mode
                    oh = pc.tile([P, P * t_c], fp16, tag="oh")
                    nc.vector.tensor_tensor(
                        out=oh[:].rearrange("p (j k) -> p j k", k=t_c),
                        in0=tg_t[:][:, None, :].broadcast_to([P, P, t_c]),
                        in1=iotexp[:].rearrange("p (j k) -> p j k", k=TCMAX)[
                            :, :, :t_c
                        ],
                        op=mybir.AluOpType.is_equal,
                    )

                    # z = tj + ti via PE (expand + identity-add); leaky via
                    # ACT Prelu (PSUM -> SBUF fp16), per PSUM group
                    zp = pb.tile([P, ts], fp16, tag="zp")
                    for g0 in range(0, t_c, EXG):
                        gn = min(EXG, t_c - g0)
                        zps = pb_ps.tile([P, EXG * P], f32, space="PSUM",
                                         tag="zps")
                        for kk in range(gn):
                            k = g0 + kk
                            # start/stop pair per 128-col chunk: PSUM group
                            # tracking is per 2KB bank; a start while another
                            # sub-bank group is pending is illegal
                            nc.tensor.matmul(
                                out=zps[:, kk * P : (kk + 1) * P],
                                lhsT=ohT[:, k * P : (k + 1) * P],
                                rhs=t2til[:],
                                start=True,
                                stop=False,
                            )
                            nc.tensor.matmul(
                                out=zps[:, kk * P : (kk + 1) * P],
                                lhsT=identb[:],
                                rhs=g1[:, k, :],
                                start=False,
                                stop=True,
                            )
                        sl_ = slice(g0 * P, (g0 + gn) * P)
                        if leaky_act:
                            # Prelu (parametric_relu) shares cayman's act
                            # table with Exp; Lrelu does not.
                            nc.scalar.activation(
                                out=zp[:, sl_],
                                in_=zps[:, : gn * P],
                                func=mybir.ActivationFunctionType.Prelu,
                                alpha=ALPHA,
                            )
                        else:
                            nc.vector.scalar_tensor_tensor(
                                out=zp[:, sl_],
                                in0=zps[:, : gn * P],
                                scalar=ALPHA,
                                in1=zps[:, : gn * P],
                                op0=mybir.AluOpType.mult,
                                op1=mybir.AluOpType.max,
                            )
                    return dict(t=t, ts=ts, t_c=t_c, g1=g1, oh=oh, zp=zp)

                def stage_b(st):
                    t_c = st["t_c"]
                    ts = st["ts"]
                    zp = st["zp"]
                    g1 = st["g1"]
                    ew = pb.tile([P, ts], fp16, tag="ew")
                    nc.vector.tensor_tensor(
                        out=ew[:].rearrange("p (k c) -> p k c", c=CH),
                        in0=zp[:].rearrange("p (k c) -> p k c", c=CH),
                        in1=a_t[:][:, None, :].broadcast_to([P, t_c, CH]),
                        op=mybir.AluOpType.mult,
                    )
                    # c=16 segment sums as a log2 tree of 2x-mode adds
                    # (TensorReduce has no 2x mode)
                    r8 = pb.tile([P, ts // 2], fp16, tag="r8")
                    wexp = pb.tile([P, t_c * P], fp16, tag="wexp")
                    r4 = wexp[:, : ts // 4]  # wexp written later; reuse early
                    lg = pb.tile([P, t_c * NH], fp16, tag="lg")
                    ew_r = ew[:].rearrange("p (k h c) -> p k h c", h=NH, c=OC)
                    r8_r = r8[:].rearrange("p (k h c) -> p k h c", h=NH, c=8)
                    r4_r = r4.rearrange("p (k h c) -> p k h c", h=NH, c=4)
                    lg_r = lg[:].rearrange("p (k h) -> p k h", h=NH)
                    with nc.allow_low_precision(reason="fp16 logit sums"):
                        nc.vector.tensor_tensor(
                            out=r8_r, in0=ew_r[:, :, :, 0:8],
                            in1=ew_r[:, :, :, 8:16], op=mybir.AluOpType.add,
                        )
                        nc.vector.tensor_tensor(
                            out=r4_r, in0=r8_r[:, :, :, 0:4],
                            in1=r8_r[:, :, :, 4:8], op=mybir.AluOpType.add,
                        )
                        r2_r = r8[:, : t_c * NH * 2].rearrange(
                            "p (k h c) -> p k h c", h=NH, c=2
                        )
                        nc.vector.tensor_tensor(
                            out=r2_r, in0=r4_r[:, :, :, 0:2],
                            in1=r4_r[:, :, :, 2:4], op=mybir.AluOpType.add,
                        )
                        r2_f = r8[:, : t_c * NH * 2].rearrange(
                            "p (k c) -> p k c", c=2 * NH
                        )
                        nc.vector.tensor_tensor(
                            out=lg_r,
                            in0=r2_f[:, :, 0 : 2 * NH : 2],
                            in1=r2_f[:, :, 1 : 2 * NH : 2],
                            op=mybir.AluOpType.add,
                        )
                    # w = exp(logits): broadcast-expanded + denominator cols
                    scat = pc.tile([P, t_c * 136], fp16, tag="scat")
                    scat_r = scat[:].rearrange("p (k c) -> p k c", c=136)
                    nc.scalar.activation(
                        out=wexp[:].rearrange("p (k h c) -> p k h c", h=NH, c=OC),
                        in_=lg_r[:, :, :, None].broadcast_to([P, t_c, NH, OC]),
                        func=mybir.ActivationFunctionType.Exp,
                    )
                    nc.scalar.activation(
                        out=scat_r[:, :, CH : CH + NH],
                        in_=lg_r,
                        func=mybir.ActivationFunctionType.Exp,
                    )
                    nc.vector.tensor_tensor(
                        out=scat_r[:, :, 0:CH],
                        in0=g1[:],
                        in1=wexp[:].rearrange("p (k c) -> p k c", c=P),
                        op=mybir.AluOpType.mult,
                    )
                    st["scat"] = scat

                def stage_c(st):
                    t = st["t"]
                    t_c = st["t_c"]
                    oh = st["oh"]
                    scat = st["scat"]
                    acc_ps = pb_ps2.tile([P, 136], f32, space="PSUM", tag="acc")
                    ohj = oh[:].rearrange("p (j k) -> p j k", k=t_c)
                    for k in range(t_c):
                        nc.tensor.matmul(
                            out=acc_ps[:],
                            lhsT=ohj[:, :, k],
                            rhs=scat[:, k * 136 : (k + 1) * 136],
                            start=(k == 0),
                            stop=(k == t_c - 1),
                        )
                    acc = pb.tile([P, 136], f32, tag="accs")
                    nc.scalar.copy(out=acc[:], in_=acc_ps[:])
                    dg = pb.tile([P, NH], f32, tag="dg")
                    nc.vector.tensor_scalar_max(
                        out=dg[:], in0=acc[:, CH : CH + NH], scalar1=1e-30
                    )
                    rc = pb.tile([P, NH], f32, tag="rc")
                    nc.vector.reciprocal(out=rc[:], in_=dg[:])
                    ot = pb.tile([P, CH], f32, tag="ot")
                    nc.vector.tensor_tensor(
                        out=ot[:].rearrange("p (h c) -> p h c", c=OC),
                        in0=acc[:, 0:CH].rearrange("p (h c) -> p h c", c=OC),
                        in1=rc[:][:, :, None].broadcast_to([P, NH, OC]),
                        op=mybir.AluOpType.mult,
                    )
                    nc.sync.dma_start(
                        out=out[t * P : (t + 1) * P, :], in_=ot[:]
                    )

                pend = []
                for t in range(nt):
                    st = stage_a(t)
                    pend.append(st)
                    if len(pend) >= 2:
                        stage_b(pend[-2])
                    if len(pend) >= 3:
                        stage_c(pend[-3])
                        pend.pop(0)
                if len(pend) >= 2:
                    stage_b(pend[-1])
                    stage_c(pend[-2])
                if pend:
                    stage_c(pend[-1])

    nc.compile()
    return nc


def kernel(x, w1, w2, a, src, tgt):
    global _last_results
    x = np.asarray(x, dtype=np.float32)
    w1 = np.asarray(w1, dtype=np.float32)
    w2 = np.asarray(w2, dtype=np.float32)
    a = np.asarray(a, dtype=np.float32)
    src = np.asarray(src)
    tgt = np.asarray(tgt)

    in_maps, dims, perm = _host_prep(x, w1, w2, a, src, tgt)
    nc = _build_program(dims)

    trace = bool(os.environ.get("KBENCH_TRACE"))
    res = bass_utils.run_bass_kernel_spmd(
        nc, in_maps, core_ids=list(range(CORES)), trace=trace
    )
    _last_results = res
    rows = [res.results[c]["out"] for c in range(CORES)]
    return _unpermute(rows, perm, dims["nslice"], dims["nt"])
